# revision 1
# baseline (speedup 1.0000x reference)
"""ADD-S (symmetric) pose loss kernel for Trainium2, 8 NeuronCores.

Sharding: data-parallel over the batch dim B=8 -> one batch element per core.
Each core computes sum_n [ min_dist(n) * conf(n) - W*log(conf(n)) ] for its
4096 points, returned as [128,1] per-partition partial sums; the host sums the
8*128 partials and divides by B*N.

Device algorithm (per core, N = 4096 points):
  1. Elementwise prologue on DVE in a SoA layout ([128 partitions, 32 free],
     point n lives at (p, f) = (n >> 5, n & 31)):
       - quat -> rotation via the unnormalized form R = M / |q|^2
       - points_model = R_gt @ (points - t_gt)
       - points_pred  = R_pred @ points_model + trans
       - aa = |points_pred|^2, bb = |points|^2 from bf16-quantized coords,
         carried as exact bf16 hi+lo row pairs (so d2 = |pp~ - q~|^2 is the
         exact squared distance of the quantized points).
  2. d2[n, m] as single-pass bf16 K=7 matmuls on the PE, packed with row
     tiling (32x128 mode, row-tile i at SBUF partition base 32i).  Work unit
     = quarter-block (query block beta of 128 points x 1024 m-columns): 2
     matmuls of 512 columns on one row-tile pair into a [128, 1024] PSUM
     tile; consecutive quarters alternate pairs {0,1}/{2,3} so 4 matmuls
     run concurrently across two in-flight quarters.
  3. The PSUM->engine drain is the kernel's true bottleneck (DVE reads PSUM
     f32 at 1 col/cycle @0.96GHz, ScalarE at 1/cycle @1.2GHz; nothing else
     on the core can read PSUM; GpSimd has no PSUM port and no min op, and
     tensor_tensor_reduce faults on HW, so folds must be DVE tensor_tensor).
     PSUM is one ring of 4 x [128, 1024] tiles (all 8 banks); ring depth 4
     hides PE refill + semaphore latency from both consumer streams:
       D-blocks: DVE tensor_reduce(min) straight from PSUM (1192ns/quarter).
       S-blocks: ScalarE copies each quarter to SBUF fp16 (1148ns/quarter),
         DVE folds the four copies with 2x-rate fp16 tensor_tensor mins +
         an in-place halving chain (2624ns/block).
     The D:S ratio (9:23) balances DVE (reduces + folds + prologue) against
     ScalarE (copies); both engines measure ~100% busy mid-stream.
  4. dist = sqrt(max(min_d2, 1e-12)); pixel = dist*clip(conf) - W*ln(clip(conf));
     per-partition row sums -> [128, 1] output.
"""

import numpy as np

B = 8
N = 4096
P = 128
F = N // P          # 32 free elems per partition in SoA layout
NB = N // P         # 32 query blocks of 128
K_DIM = 7
W_RATE = 0.015
SYM_CLASS_IDS = {1}

N_D_BLOCKS = 9      # blocks consumed directly by DVE from PSUM

_cache = {}


def _np_f32(x):
    return np.ascontiguousarray(np.asarray(x), dtype=np.float32)


def _block_schedule():
    """32 blocks, N_D_BLOCKS of type 'D' spread evenly among the 'S'."""
    sched = []
    acc = 0
    for beta in range(NB):
        acc += N_D_BLOCKS
        if acc >= NB:
            acc -= NB
            sched.append((beta, "D"))
        else:
            sched.append((beta, "S"))
    if sched[-1][1] == "D":
        for k in range(NB - 2, -1, -1):
            if sched[k][1] == "S":
                sched[k] = (sched[k][0], "D")
                sched[-1] = (sched[-1][0], "S")
                break
    assert sum(1 for _, t in sched if t == "D") == N_D_BLOCKS
    return sched


def _emit(ctx, tc, out_ap, ins):
    import concourse.bass as bass
    from concourse import mybir

    nc = tc.nc
    f32 = mybir.dt.float32
    Alu = mybir.AluOpType
    Act = mybir.ActivationFunctionType
    X = mybir.AxisListType.X

    quat, trans, conf, pose, points = (
        ins["pred_quat"], ins["pred_trans"], ins["pred_conf"],
        ins["pose"], ins["points"],
    )

    pool = ctx.enter_context(tc.tile_pool(name="main", bufs=1))

    def t(tag, shape, dtype=f32):
        return pool.tile(shape, dtype, tag=tag, name=tag)

    dma = nc.sync.dma_start

    # ---------------- input loads ----------------
    q_t = t("q_t", [P, F * 4])       # quat rows, 4 per point
    p_t = t("p_t", [P, F * 3])       # points
    tr_t = t("tr_t", [P, F * 3])     # pred_trans
    bc = t("bc", [P, 12])            # pose scalars broadcast across partitions
    conf_b = t("conf_b", [P, NB])    # conf in output (SoA-B) order

    nc.gpsimd.dma_start(out=q_t, in_=quat.rearrange("(p f) c -> p (f c)", p=P))
    nc.sync.dma_start(out=p_t, in_=points.rearrange("(p f) c -> p (f c)", p=P))
    nc.gpsimd.dma_start(out=tr_t, in_=trans.rearrange("(p f) c -> p (f c)", p=P))
    nc.sync.dma_start(out=bc, in_=bass.AP(tensor=pose.tensor,
                                          offset=pose.offset,
                                          ap=[[0, P], [1, 12]]))

    q3 = q_t.rearrange("p (f c) -> p f c", c=4)
    p3 = p_t.rearrange("p (f c) -> p f c", c=3)
    tr3 = tr_t.rearrange("p (f c) -> p f c", c=3)

    vec = nc.vector
    from concourse.tile import add_dep_helper

    # DMA-wait funnel: a chain of TT ops absorbs every input-DMA semaphore
    # wait (1 per instruction) so downstream TensorScalar ops, which have
    # very few HW sync-wait slots, never carry DMA waits themselves.
    scrf = t("scrf", [P, 1])
    vec.tensor_copy(out=scrf, in_=q_t[:, 0:1])
    for dep_t in (p_t, tr_t, bc):
        last_f = vec.tensor_tensor(out=scrf, in0=scrf, in1=dep_t[:, 0:1],
                                   op=Alu.add)

    def pin(inst):
        add_dep_helper(inst.ins, last_f.ins, sync=False,
                       reason="order after input-DMA funnel")
        return inst

    # ---------------- quaternion -> unnormalized rotation ----------------
    sq = t("sq", [P, F * 4])
    pin(vec.tensor_tensor(out=sq, in0=q_t, in1=q_t, op=Alu.mult))
    sq3 = sq.rearrange("p (f c) -> p f c", c=4)
    s2 = t("s2", [P, F])
    vec.reduce_sum(s2, sq3, axis=X)
    s2c = t("s2c", [P, F])
    vec.tensor_scalar_max(s2c, s2, 1e-16)
    rec = t("rec", [P, F])
    vec.reciprocal(rec, s2c)

    # gt transform: pm_k = sum_j Rg[k,j] * (points_j - t_j)
    gp = nc.gpsimd
    pc = [t(f"pc{j}", [P, F]) for j in range(3)]
    for j in range(3):
        pin(vec.tensor_scalar_sub(pc[j], p3[:, :, j],
                                  bc[:, 4 * j + 3: 4 * j + 4]))
    pm_cat = t("pm_cat", [P, F * 3])      # [p, f, j] (j innermost)
    pm_cat3 = pm_cat.rearrange("p (f j) -> p f j", j=3)
    pm = [pm_cat3[:, :, k] for k in range(3)]
    for k in range(3):
        pin(vec.tensor_scalar_mul(pm[k], pc[0], bc[:, 4 * k: 4 * k + 1]))
        vec.scalar_tensor_tensor(out=pm[k], in0=pc[1],
                                 scalar=bc[:, 4 * k + 1: 4 * k + 2],
                                 in1=pm[k], op0=Alu.mult, op1=Alu.add)
        vec.scalar_tensor_tensor(out=pm[k], in0=pc[2],
                                 scalar=bc[:, 4 * k + 2: 4 * k + 3],
                                 in1=pm[k], op0=Alu.mult, op1=Alu.add)

    qw, qx, qy, qz = (q3[:, :, 0], q3[:, :, 1], q3[:, :, 2], q3[:, :, 3])
    xx, yy, zz = (sq3[:, :, 1], sq3[:, :, 2], sq3[:, :, 3])

    # all six q-pair products from three shifted-view wide multiplies:
    # P1 = q[c]*q[c+1] -> (wx, xy, yz); P2 = q[c]*q[c+2] -> (wy, xz);
    # P3 = w*z.  Scaled by 2 in place (fp32 tensor_scalar runs 2x).
    P1 = t("P1", [P, F * 3])
    P2 = t("P2", [P, F * 2])
    P3 = t("P3", [P, F])
    pin(vec.tensor_tensor(out=P1.rearrange("p (f c) -> p f c", c=3),
                          in0=q3[:, :, 0:3], in1=q3[:, :, 1:4], op=Alu.mult))
    pin(vec.tensor_tensor(out=P2.rearrange("p (f c) -> p f c", c=2),
                          in0=q3[:, :, 0:2], in1=q3[:, :, 2:4], op=Alu.mult))
    pin(vec.tensor_tensor(out=P3, in0=q3[:, :, 0], in1=q3[:, :, 3],
                          op=Alu.mult))
    vec.tensor_scalar_mul(P1, P1, 2.0)
    vec.tensor_scalar_mul(P2, P2, 2.0)
    vec.tensor_scalar_mul(P3, P3, 2.0)
    P1v = P1.rearrange("p (f c) -> p f c", c=3)
    P2v = P2.rearrange("p (f c) -> p f c", c=2)
    wx2, xy2, yz2 = P1v[:, :, 0], P1v[:, :, 1], P1v[:, :, 2]
    wy2, xz2 = P2v[:, :, 0], P2v[:, :, 1]
    wz2 = P3

    def tt(tag, a, b_, op):
        o = t(tag, [P, F])
        vec.tensor_tensor(out=o, in0=a, in1=b_, op=op)
        return o

    def tt_into(out, a, b_, op):
        vec.tensor_tensor(out=out, in0=a, in1=b_, op=op)

    # rotation-matrix entries written into M9 [p, row, f, j] (j innermost)
    # so points_pred falls out of ONE wide multiply + ONE innermost reduce:
    #   vv[p,r,f,j] = M9[p,r,f,j] * pm[p,f,j];  v[p,r,f] = sum_j vv
    M9 = t("M9", [P, 3 * F * 3])
    M9v = M9.rearrange("p (r f j) -> p r f j", r=3, j=3)

    def m9(r, j):
        return M9v[:, r, :, j]

    tt_into(m9(0, 1), xy2, wz2, Alu.subtract)   # M[0][1]
    tt_into(m9(1, 0), xy2, wz2, Alu.add)        # M[1][0]
    tt_into(m9(0, 2), xz2, wy2, Alu.add)        # M[0][2]
    tt_into(m9(2, 0), xz2, wy2, Alu.subtract)   # M[2][0]
    tt_into(m9(1, 2), yz2, wx2, Alu.subtract)   # M[1][2]
    tt_into(m9(2, 1), yz2, wx2, Alu.add)        # M[2][1]

    a0 = tt("a0", yy, zz, Alu.add)
    a1 = tt("a1", xx, zz, Alu.add)
    a2 = tt("a2", xx, yy, Alu.add)
    for k, ak in enumerate((a0, a1, a2)):
        vec.scalar_tensor_tensor(out=m9(k, k), in0=ak, scalar=-2.0, in1=s2,
                                 op0=Alu.mult, op1=Alu.add)

    pm_rep = t("pm_rep", [P, 3 * F * 3])
    for r in range(3):
        vec.tensor_copy(out=pm_rep[:, r * F * 3:(r + 1) * F * 3], in_=pm_cat)
    vv = t("vv", [P, 3 * F * 3])
    vec.tensor_tensor(out=vv, in0=M9, in1=pm_rep, op=Alu.mult)
    v_cat = t("v_cat", [P, 3 * F])
    vec.tensor_reduce(v_cat, vv.rearrange("p (rf j) -> p rf j", j=3), axis=X,
                      op=Alu.add)
    rec_rep = t("rec_rep", [P, 3 * F])
    for r in range(3):
        vec.tensor_copy(out=rec_rep[:, r * F:(r + 1) * F], in_=rec)
    vec.tensor_tensor(out=v_cat, in0=v_cat, in1=rec_rep, op=Alu.mult)
    pp_cat = t("pp_cat", [P, 3 * F])
    vec.tensor_tensor(out=pp_cat, in0=v_cat,
                      in1=tr_t.rearrange("p (f c) -> p c f", c=3), op=Alu.add)
    pp = [pp_cat[:, r * F:(r + 1) * F] for r in range(3)]

    bf16 = mybir.dt.bfloat16

    # quantize predicted points to bf16; aa computed in f32 FROM the
    # quantized coords, then split into an exact bf16 hi+lo pair.
    # ppq coords stored [P, 3, F] (coord-major: row i contiguous) so the
    # -2x scale and |pp~|^2 square are single wide ops and each row stays
    # a contiguous DMA source.
    ppq_cat = t("ppq_cat", [P, 3 * F], bf16)
    n2_cat = t("n2_cat", [P, 3 * F], bf16)
    for i in range(3):
        vec.tensor_copy(out=ppq_cat[:, i * F:(i + 1) * F], in_=pp[i])
    vec.tensor_scalar_mul(n2_cat, ppq_cat, -2.0)   # exact: *2 and bf16 input
    ppq = [ppq_cat[:, i * F:(i + 1) * F] for i in range(3)]
    n2 = [n2_cat[:, i * F:(i + 1) * F] for i in range(3)]
    sq_cat = t("sq_cat", [P, 3 * F])
    vec.tensor_tensor(out=sq_cat, in0=ppq_cat, in1=ppq_cat, op=Alu.mult)
    aa = t("aa", [P, F])
    vec.tensor_reduce(aa, sq_cat.rearrange("p (i f) -> p f i", i=3), axis=X,
                      op=Alu.add)
    aa_h = t("aa_h", [P, F], bf16)
    vec.tensor_copy(out=aa_h, in_=aa)
    aa_l = t("aa_l", [P, F], bf16)
    vec.tensor_tensor(out=aa_l, in0=aa, in1=aa_h, op=Alu.subtract)

    # quantized target coords + bb = |q~|^2 as exact hi+lo pair (GpSimd)
    gscr = t("gscr", [P, F])
    pcomp = []
    for j in range(3):
        o = t(f"pcomp{j}", [P, F], bf16)
        gp.tensor_copy(out=o, in_=p3[:, :, j])
        pcomp.append(o)
    bb = t("bb", [P, F])
    gp.tensor_tensor(out=bb, in0=pcomp[0], in1=pcomp[0], op=Alu.mult)
    gp.tensor_tensor(out=gscr, in0=pcomp[1], in1=pcomp[1], op=Alu.mult)
    gp.tensor_tensor(out=bb, in0=bb, in1=gscr, op=Alu.add)
    gp.tensor_tensor(out=gscr, in0=pcomp[2], in1=pcomp[2], op=Alu.mult)
    gp.tensor_tensor(out=bb, in0=bb, in1=gscr, op=Alu.add)
    bb_h = t("bb_h", [P, F], bf16)
    gp.tensor_copy(out=bb_h, in_=bb)
    bb_l = t("bb_l", [P, F], bf16)
    gp.tensor_tensor(out=bb_l, in0=bb, in1=bb_h, op=Alu.subtract)

    # conf gather in block order: conf_b[p, beta] = conf[beta*128 + p]
    gp.dma_start(out=conf_b, in_=bass.AP(tensor=conf.tensor,
                                         offset=conf.offset,
                                         ap=[[1, P], [P, NB]]))

    # ---------------- matmul operands ----------------
    # lhsT/rhs: logical rows 0..6 at partitions 0..6 (contiguous, identity
    # flatten n = p*32 + f), replicated to partition bases 32/64/96 so each
    # PE row tile streams from its own partitions.
    lhsT = t("lhsT", [P, N], bf16)
    rhs = t("rhs", [P, N], bf16)
    ones_t = t("ones_t", [P, F], bf16)
    vec.memset(ones_t, 1.0)
    dma(out=lhsT[2:3, :], in_=ones_t)
    dma(out=lhsT[3:4, :], in_=ones_t)
    gp.dma_start(out=rhs[0:1, :], in_=ones_t)
    gp.dma_start(out=rhs[1:2, :], in_=ones_t)
    # lhsT rows split across queues: n2 on sync, aa on scalar (2-3 serial
    # DMAs per queue instead of 5), replications after the aa rows on the
    # scalar queue; rhs rows + replications on the gpsimd queue.
    for r, row in ((4, n2[0]), (5, n2[1]), (6, n2[2])):
        dma(out=lhsT[r:r + 1, :], in_=row)
    nc.scalar.dma_start(out=lhsT[0:1, :], in_=aa_h)
    nc.scalar.dma_start(out=lhsT[1:2, :], in_=aa_l)
    for r, row in ((2, bb_h), (3, bb_l),
                   (4, pcomp[0]), (5, pcomp[1]), (6, pcomp[2])):
        gp.dma_start(out=rhs[r:r + 1, :], in_=row)
    for base in (32, 64, 96):
        nc.scalar.dma_start(out=lhsT[base:base + K_DIM, :],
                            in_=lhsT[0:K_DIM, :])
        gp.dma_start(out=rhs[base:base + K_DIM, :], in_=rhs[0:K_DIM, :])

    # ---------------- conf term (early: ACT Ln table load overlaps) -------
    cc = t("cc", [P, NB])
    pin(vec.tensor_scalar_max(cc, conf_b, 1e-4))
    vec.tensor_scalar_min(cc, cc, 1.0)
    lnc = t("lnc", [P, NB])
    ln_inst = nc.scalar.activation(lnc, cc, Act.Ln)
    # prefetch the sqrt table set during the main loop (after all Ln uses)
    sq_pre = t("sq_pre", [P, 1])
    sq_inst = nc.scalar.sqrt(sq_pre, cc[:, 0:1])
    add_dep_helper(sq_inst.ins, ln_inst.ins, sync=False,
                   reason="load sqrt ACT table after ln")


    # ---------------- main loop: quarter-block units, two consumer streams
    # PSUM: one ring of 4 x [128, 1024] f32 tiles (2 banks each = all 8
    # banks).  Each quarter = 2 matmuls of 512 cols on one row-tile pair;
    # consecutive quarters alternate pairs {0,1}/{2,3} so 4 matmuls run
    # concurrently across two in-flight quarters.  Ring depth 4 gives the
    # PE ~4 consumer-slots of headroom, hiding refill + semaphore latency.
    fp16 = mybir.dt.float16
    QC = 1024
    minsb_q = [t(f"minsb_{q}", [P, NB]) for q in range(4)]
    for q in range(1, 4):
        vec.memset(minsb_q[q], 3e38)   # S-blocks only write minsb_q[0]

    ps = ctx.enter_context(tc.tile_pool(name="ps", bufs=4, space="PSUM"))
    scp = ctx.enter_context(tc.tile_pool(name="scp", bufs=6))
    fpool = ctx.enter_context(tc.tile_pool(name="fold", bufs=3))

    qidx = 0
    for beta, typ in _block_schedule():
        cps = []
        fa = fb = None
        for qh in range(4):
            ptile = ps.tile([P, QC], f32, tag="ps", name=f"ps_{beta}_{qh}")
            pair = qidx % 2
            qidx += 1
            for ii in range(2):
                i = 2 * pair + ii
                nc.tensor.matmul(
                    ptile[:, ii * 512:(ii + 1) * 512],
                    lhsT[32 * i: 32 * i + K_DIM,
                         beta * P:(beta + 1) * P],
                    rhs[32 * i: 32 * i + K_DIM,
                        qh * QC + ii * 512: qh * QC + (ii + 1) * 512],
                    start=True, stop=True,
                    tile_position=(32 * i, 0),
                )
            if typ == "D":
                vec.tensor_reduce(minsb_q[qh][:, beta:beta + 1], ptile,
                                  axis=X, op=Alu.min)
            else:
                cp = scp.tile([P, QC], fp16, tag="cp",
                              name=f"cp_{beta}_{qh}")
                nc.scalar.copy(out=cp, in_=ptile)
                cps.append(cp)
                if qh == 1:
                    fa = fpool.tile([P, QC], fp16, tag="fold",
                                    name=f"fa_{beta}")
                    vec.tensor_tensor(out=fa, in0=cps[0], in1=cps[1],
                                      op=Alu.min)
        if typ == "S":
            fb = fpool.tile([P, QC], fp16, tag="fold", name=f"fb_{beta}")
            vec.tensor_tensor(out=fb, in0=cps[2], in1=cps[3], op=Alu.min)
            vec.tensor_tensor(out=fa, in0=fa, in1=fb, op=Alu.min)
            w = QC // 2
            while w >= 128:
                vec.tensor_tensor(out=fa[:, 0:w], in0=fa[:, 0:w],
                                  in1=fa[:, w:2 * w], op=Alu.min)
                w //= 2
            vec.tensor_reduce(minsb_q[0][:, beta:beta + 1], fa[:, 0:128],
                              axis=X, op=Alu.min)

    # ---------------- tail: combine quarters, sqrt, pixel loss, row sums --
    minsb = t("minsb", [P, NB])
    scr2 = t("scr2", [P, NB])
    vec.tensor_tensor(out=minsb, in0=minsb_q[0], in1=minsb_q[1], op=Alu.min)
    vec.tensor_tensor(out=scr2, in0=minsb_q[2], in1=minsb_q[3], op=Alu.min)
    vec.tensor_tensor(out=minsb, in0=minsb, in1=scr2, op=Alu.min)
    vec.tensor_scalar_max(minsb, minsb, 1e-12)
    dist = t("dist", [P, NB])
    nc.scalar.sqrt(dist, minsb)

    pix = t("pix", [P, NB])
    vec.tensor_tensor(out=pix, in0=dist, in1=cc, op=Alu.mult)
    vec.scalar_tensor_tensor(out=pix, in0=lnc, scalar=-W_RATE, in1=pix,
                             op0=Alu.mult, op1=Alu.add)
    sums = t("sums", [P, 1])
    vec.reduce_sum(sums, pix, axis=X)
    dma(out=out_ap, in_=sums)


def _build():
    from contextlib import ExitStack

    import concourse.bacc as bacc
    import concourse.tile as tile
    from concourse import mybir

    f32 = mybir.dt.float32
    nc = bacc.Bacc("TRN2", debug=False, enable_asserts=False, num_devices=B)
    ins = {
        "pred_quat": nc.dram_tensor("pred_quat", [N, 4], f32,
                                    kind="ExternalInput").ap(),
        "pred_trans": nc.dram_tensor("pred_trans", [N, 3], f32,
                                     kind="ExternalInput").ap(),
        "pred_conf": nc.dram_tensor("pred_conf", [N, 1], f32,
                                    kind="ExternalInput").ap(),
        "pose": nc.dram_tensor("pose", [3, 4], f32, kind="ExternalInput").ap(),
        "points": nc.dram_tensor("points", [N, 3], f32,
                                 kind="ExternalInput").ap(),
    }
    out_ap = nc.dram_tensor("out_sums", [P, 1], f32, kind="ExternalOutput").ap()
    with tile.TileContext(nc) as tc:
        with ExitStack() as ctx:
            _emit(ctx, tc, out_ap, ins)
    nc.compile()
    return nc


def _get_nc():
    if "nc" not in _cache:
        _cache["nc"] = _build()
    return _cache["nc"]


def _numpy_reference(pred_quat, pred_trans, pred_conf, pose, points, cls_id):
    """Full-precision numpy fallback (used only for the non-symmetric branch)."""
    q = pred_quat.astype(np.float64)
    q = q / np.clip(np.linalg.norm(q, axis=-1, keepdims=True), 1e-8, None)
    w, x, y, z = q[..., 0], q[..., 1], q[..., 2], q[..., 3]
    r = np.stack([
        1 - 2 * (y * y + z * z), 2 * (x * y - w * z), 2 * (x * z + w * y),
        2 * (x * y + w * z), 1 - 2 * (x * x + z * z), 2 * (y * z - w * x),
        2 * (x * z - w * y), 2 * (y * z + w * x), 1 - 2 * (x * x + y * y),
    ], axis=-1).reshape(q.shape[:-1] + (3, 3))
    gt_r = pose[:, :3, :3].astype(np.float64)
    gt_t = pose[:, :3, 3].astype(np.float64)
    pc = points.astype(np.float64) - gt_t[:, None, :]
    pm = np.einsum("bkj,bnj->bnk", gt_r, pc)
    ppred = np.einsum("bnij,bnj->bni", r, pm) + pred_trans.astype(np.float64)
    tgt = points.astype(np.float64)
    if int(cls_id[0]) in SYM_CLASS_IDS:
        aa = np.sum(ppred * ppred, axis=-1)
        bb2 = np.sum(tgt * tgt, axis=-1)
        ab = np.einsum("bnd,bmd->bnm", ppred, tgt)
        d2 = aa[:, :, None] + bb2[:, None, :] - 2.0 * ab
        loss_dist = np.sqrt(np.maximum(d2, 1e-12)).min(axis=2)
    else:
        loss_dist = np.linalg.norm(ppred - tgt, axis=2)
    c = np.clip(pred_conf[..., 0].astype(np.float64), 1e-4, 1.0)
    return np.float32(np.mean(loss_dist * c - W_RATE * np.log(c)))


def kernel(pred_quat, pred_trans, pred_conf, pose, points, cls_id):
    pred_quat = _np_f32(pred_quat)
    pred_trans = _np_f32(pred_trans)
    pred_conf = _np_f32(pred_conf)
    pose = _np_f32(pose)
    points = _np_f32(points)
    cls_id = np.asarray(cls_id)

    assert pred_quat.shape == (B, N, 4), pred_quat.shape

    if int(cls_id[0]) not in SYM_CLASS_IDS:
        return np.array(
            _numpy_reference(pred_quat, pred_trans, pred_conf, pose, points,
                             cls_id),
            dtype=np.float32)

    from concourse.bass_utils import run_bass_kernel_spmd

    nc = _get_nc()
    in_maps = [
        {
            "pred_quat": np.ascontiguousarray(pred_quat[c]),
            "pred_trans": np.ascontiguousarray(pred_trans[c]),
            "pred_conf": np.ascontiguousarray(pred_conf[c]),
            "pose": np.ascontiguousarray(pose[c]),
            "points": np.ascontiguousarray(points[c]),
        }
        for c in range(B)
    ]
    res = run_bass_kernel_spmd(nc, in_maps, core_ids=list(range(B)))
    total = np.float64(0.0)
    for r in res.results:
        total += np.sum(r["out_sums"].astype(np.float64))
    return np.array(total / (B * N), dtype=np.float32)

